# revision 7
# baseline (speedup 1.0000x reference)
"""Trainium2 Bass kernel v8 for nn_MappingNetwork (histogram_binning).

seed = trunc((z + 1e5) * C), C = (1e8-1)/2e5 in f32; out[r, 0:512] = seed[r].

No input DMACopy. The only iota-built index is the tiny 8-slot gather
index: the gather/scatter ucode on queue 1 runs on Q7 cpu pair 2/3,
whose RX/TX index stripes live at partitions 32:48 / 48:64 -- one legal
[32:64] iota covers both, and its inherent +16 RX->TX offset becomes a
+16 source-row shift absorbed by the host pack (partitions 0:16 get a
matching base+16 iota so the CoreSim executor agrees with hardware).
Everything else -- the z column, the 2048-entry scatter index table,
and the leftover-row source -- is host-packed into one [144, 192] f32
input and pulled into SBUF by three cheap gathers (elem 64, step 192).

Pool (serial): memset+iota idxg -> gather idx-table -> gather z
(x4-replicated) -> tensor_scalar (x2 in-broadcast -> svK [p,128] int32)
-> scatterA (rows 0:1024) -> scatterB (rows 1024:2048) into out[:, 0:8]
(output buffers arrive zeroed: PJRT donates zero buffers, the native
path pre-zeros) -> gather leftover src -> tensor_scalar -> kv_writeback
rows {1023, 2047} (batch 2, d_head 512, ctx 0).
Act amp rows 0:1023, SP amp rows 1024:2047: full-row broadcast DMA
out[:, :] <- out[:, 0:8] x64 (c=0 chunk self-copies; full contiguity
keeps the 500ns cost floor, row*chunk merge 65472/65408 < 65536).
"""

import numpy as np

N_CORES = 8
B = 16384
W = 512
ROWS = B // N_CORES  # 2048
P = 128
SH = 16              # queue-1 RX->TX iota offset = gather source-row shift
NS = ROWS // P       # 16 slots
REP = 8              # seed columns K
K = REP
GC = 64              # gather elem (f32)
NCOL = 3 * GC        # 192 input cols: z | idx-table | leftover
ZROWS = P + SH       # 144 gather-source rows
HALF = ROWS // 2
ROW_L0 = HALF - 1    # leftover row 1023
ROW_L1 = ROWS - 1    # leftover row 2047

CONST = float(np.float32(499.999995))

_nc_cache = {}


def build_nc():
    if "nc" in _nc_cache:
        return _nc_cache["nc"]
    import concourse.bass as bass
    import concourse.mybir as mybir

    dt = mybir.dt
    alu = mybir.AluOpType

    nc = bass.Bass(detect_race_conditions=False, num_swdge_queues=2)

    entry = nc.m.functions[0].blocks[0]
    entry.instructions[:] = [
        i
        for i in entry.instructions
        if type(i).__name__ not in ("InstMemset", "InstDrain", "InstEventSemaphore")
    ]

    zin = nc.dram_tensor("zin", [ZROWS, NCOL], dt.float32, kind="ExternalInput")
    out = nc.dram_tensor("out", [ROWS, W], dt.int32, kind="ExternalOutput")

    from contextlib import ExitStack

    es = ExitStack()
    with es:
        idxg = es.enter_context(nc.sbuf_tensor("idxg", [P, 8], dt.int16))
        idxt = es.enter_context(nc.sbuf_tensor("idxt", [P, GC], dt.float32))
        zv = es.enter_context(nc.sbuf_tensor("zv", [P, GC], dt.float32))
        zvL = es.enter_context(nc.sbuf_tensor("zvL", [P, GC], dt.float32))
        svK = es.enter_context(nc.sbuf_tensor("svK", [P, P], dt.int32))
        svL = es.enter_context(nc.sbuf_tensor("svL", [P, 8], dt.int32))
        gt_sem = es.enter_context(nc.semaphore("gt_sem"))
        g1_sem = es.enter_context(nc.semaphore("g1_sem"))
        g2_sem = es.enter_context(nc.semaphore("g2_sem"))
        sa_sem = es.enter_context(nc.semaphore("sa_sem"))
        sb_sem = es.enter_context(nc.semaphore("sb_sem"))
        kv_sem = es.enter_context(nc.semaphore("kv_sem"))
        a_sem = es.enter_context(nc.semaphore("a_sem"))
        b_sem = es.enter_context(nc.semaphore("b_sem"))

        block = bass.BassBlock(nc, "main")
        block.__enter__()

        def amp_rows(eng, lo, hi, sem):
            eng.dma_start(
                out=out[lo:hi, :].rearrange("r (c k) -> r c k", k=K),
                in_=out[lo:hi, 0:K].unsqueeze(1).broadcast_to([hi - lo, W // K, K]),
            ).then_inc(sem, 16)

        @block.gpsimd
        def _(pool):
            with pool.register("ngr") as ngr, pool.register("nsr") as nsr:
                pool.reg_mov(ngr, P)
                pool.reg_mov(nsr, HALF)
                from concourse import library_config

                # gather idx: executor stripe [0:16] = t+16; queue-1 hw
                # stripes rx [32:48] = t, tx [48:64] = t+16. memset first:
                # the interp requires the full [128, 8] view initialized.
                pool.memset(idxg[:, :], 0)
                pool.iota(idxg[0:16, 0:8], [[16, 8]], base=SH, channel_multiplier=1)
                pool.iota(idxg[32:64, 0:8], [[16, 8]], base=0, channel_multiplier=1)
                pool.load_library(library_config.attnmlp)

                def gat(dst, col, sem):
                    pool.dma_gather(
                        out_ap=dst[:, :].unsqueeze(1),
                        in_ap=zin[:, col : col + GC],
                        idxs_ap=idxg[:, :],
                        num_idxs=P,
                        num_idxs_reg=ngr,
                        elem_size=GC,
                        elem_step=NCOL,
                        queue_num=1,
                    ).then_inc(sem, 16)

                gat(idxt, GC, gt_sem)
                gat(zv, 0, g1_sem)
                pool.wait_ge(g1_sem, 16)
                nc.gpsimd.tensor_scalar(
                    svK[:, :].rearrange("p (s r j) -> p s r j", r=2, j=4),
                    zv[:, :]
                    .rearrange("p (s j) -> p s j", j=4)
                    .unsqueeze(2)
                    .broadcast_to([P, NS, 2, 4]),
                    100000.0,
                    CONST,
                    alu.add,
                    alu.mult,
                )
                pool.wait_ge(gt_sem, 16)
                sv3 = svK[:, :].rearrange("p (s j) -> p s j", j=REP)
                idxt16 = idxt[:, :].bitcast(dt.int16)
                for half, sem in ((0, sa_sem), (1, sb_sem)):
                    pool.dma_scatter_add(
                        out_ap=out[:, 0:K],
                        in_ap=sv3[:, 8 * half : 8 * (half + 1)],
                        idxs_ap=idxt16[:, 64 * half : 64 * (half + 1)],
                        num_idxs=HALF,
                        num_idxs_reg=nsr,
                        elem_size=K,
                        elem_step=W,
                        queue_num=1,
                    ).then_inc(sem, 16)
                # leftover rows 1023 / 2047 via kv_writeback (batch 2)
                gat(zvL, 2 * GC, g2_sem)
                pool.wait_ge(g2_sem, 16)
                nc.gpsimd.tensor_scalar(
                    svL[:, :], zvL[:, 0:8], 100000.0, CONST, alu.add, alu.mult
                )
                pool.kv_writeback(
                    out_ap=bass.AP(
                        out,
                        ROW_L0 * W,
                        [[(ROW_L1 - ROW_L0) * W, 2], [4, 128], [1, 4], [1, 1]],
                    ),
                    in_ap=svL[:, :].rearrange("p (d b) -> p d b", b=2).unsqueeze(-1),
                    ctx_idxs_ap=zvL[:, 8:10].bitcast(dt.int32),
                    queue_num=1,
                ).then_inc(kv_sem, 16)
                pool.wait_ge(sa_sem, 16)
                pool.wait_ge(sb_sem, 16)
                pool.wait_ge(kv_sem, 16)

        @block.scalar
        def _(act):
            act.wait_ge(sa_sem, 16)
            amp_rows(act, 0, ROW_L0, a_sem)
            act.wait_ge(a_sem, 16)

        @block.sync
        def _(sp):
            sp.wait_ge(sb_sem, 16)
            amp_rows(sp, HALF, ROW_L1, b_sem)
            sp.wait_ge(b_sem, 16)

        for engine, last_body in block.last_body.items():
            with nc.body(last_body, parent=nc.cur_bb, allow_existing_parent=True):
                engine.br(block.end_bb)
        nc.switch_bb(block.end_bb)
        nc.cur_block = None

    from concourse.library_overlay import lower_extended_insts

    lower_extended_insts(nc)

    _nc_cache["nc"] = nc
    return nc


def pack_zin(zslice):
    """zslice: [2048] f32 for one core -> [144, 192] f32 gather source.

    Row 16+p: cols 0:64    [z[128 s + p] x4 for s in 0..15]
              cols 64:128  int16 idx[p, m] = (p%16) + 16 m  (m < 128)
              cols 128:192 [z[1023], z[2047]] x4, rest pad
    """
    zin = np.zeros((ZROWS, NCOL), dtype=np.float32)
    zg = zslice.reshape(NS, P).T  # [p, s]
    zin[SH:, 0:GC] = np.repeat(zg, 4, axis=1)
    idx = (
        (np.arange(P)[:, None] % 16) + 16 * np.arange(P)[None, :]
    ).astype(np.int16)  # [128, 128]
    zin[SH:, GC : 2 * GC] = idx.view(np.float32)
    pair = np.array([zslice[ROW_L0], zslice[ROW_L1]], dtype=np.float32)
    zin[SH:, 2 * GC : 2 * GC + 8] = np.tile(pair, 4)[None, :]
    return np.ascontiguousarray(zin)


def kernel(z, c=None, **_unused):
    z = np.ascontiguousarray(np.asarray(z), dtype=np.float32)
    assert z.shape == (B, W), z.shape
    nc = build_nc()
    from concourse.bass_utils import run_bass_kernel_spmd

    in_maps = []
    for i in range(N_CORES):
        in_maps.append({"zin": pack_zin(z[i * ROWS : (i + 1) * ROWS, 0].copy())})
    for attempt in range(3):
        res = run_bass_kernel_spmd(nc, in_maps, core_ids=list(range(N_CORES)))
        globals()["LAST_RESULT"] = res
        full = np.concatenate([r["out"] for r in res.results], axis=0).astype(np.int32)
        # A transiently failed execution hands back the donated zero output
        # buffers (fully, or partially if only one async writer failed); a
        # true seed of 0 would need z <= -1e5, impossible for any finite
        # randn input, so any zero anywhere means "retry".
        if not (full == 0).any():
            break
    return full
